# revision 10
# baseline (speedup 1.0000x reference)
"""Trainium2 Bass kernel for CrossAttention (B=4, QL=KL=2048, D=1024, fp32).

reference:
    query = hidden_states @ Wq                      # [B, QL, D]
    kv    = decoder_hidden_states @ Wkv             # [B, KL, 2D]
    key, value = split(kv, 2, axis=-1)
    scores = einsum('bqd,bkd->bqk', query, key) / sqrt(D)
    w = softmax(scores, axis=-1)
    out = einsum('bqk,bkd->bqd', w, value)          # [B, QL, D]

Sharding: 8 cores = batch(4) x q-half(2).  Each core owns 1024 query rows of
one batch and computes the full K/V projection for its batch (KV work
duplicated x2 across the pair sharing a batch; no collectives needed).

All matmuls run in float32r (TF32-like), which streams at full PE rate for
moving dims >= 256.  Softmax runs without max-subtraction (scores here are
~N(0,1); exp stays far from fp32 limits) using ACT's fused exp(scale*x) with
accum_out row sums.

Phase order QT -> KT -> V -> attention, with weight pools opened up-front and
closed manually mid-kernel so every phase's inputs prefetch during the
previous phase's compute (SBUF is too small to hold everything at once).

All DRAM operands are passed in block layout [nblk, 128, DS*128] so each DMA
is a [128, 4KB-row] transfer (efficient descriptors).

This walrus build allows only ONE embedded semaphore wait per hardware
instruction; legalize_waits() splits any extra waits onto injected
same-engine NOPs after Tile scheduling.
"""

import sys

if "/opt/trn_rl_repo" not in sys.path:
    sys.path.insert(0, "/opt/trn_rl_repo")

import numpy as np

import bass_rust
import concourse.bass as bass
import concourse.mybir as mybir
import concourse.tile as tile
from concourse.bass_utils import run_bass_kernel_spmd

F32 = mybir.dt.float32
F32R = mybir.dt.float32r
EXP = mybir.ActivationFunctionType.Exp

N_CORES = 8
B, QL, KL, D = 4, 2048, 2048, 1024


def legalize_waits(nc, max_waits=1):
    """TRN2 instructions embed at most one semaphore wait.  Move excess waits
    emitted by Tile onto same-engine NOPs inserted just before the owning
    instruction (engine FIFO makes this semantically identical)."""
    cnt = 0
    for fn in nc.m.functions:
        for bb in fn.blocks:
            out = []
            changed = False
            for ins in bb.instructions:
                si = ins.sync_info
                if si is not None and si.on_wait and len(si.on_wait) > max_waits:
                    waits = list(si.on_wait)
                    for w in waits[:-max_waits]:
                        cnt += 1
                        nop = bass_rust.InstNoOp(name=f"I-wfix-{cnt}")
                        nop.engine = ins.engine
                        nop.sync_info = mybir.SyncInfo(on_wait=[w], on_update=[])
                        out.append(nop)
                    ins.sync_info = mybir.SyncInfo(
                        on_wait=waits[-max_waits:],
                        on_update=list(si.on_update or []),
                    )
                    changed = True
                out.append(ins)
            if changed:
                bb.instructions = out
    return cnt


def build_attention(nc, QS, KLp, Dp, scale):
    DS = Dp // 128          # contraction subtiles
    NDO = Dp // 128         # output-d 128-chunks
    NKC = KLp // 512        # k 512-chunks (scores)
    NKT = KLp // 128        # k 128-chunks
    NQT = QS // 128         # q tiles
    NDC = Dp // 512         # d 512-chunks (AV / Wkv_hi)
    NA1 = KLp // 512        # A1 rhs 512-chunks
    NQC = QS // 512         # B rhs 512-chunks
    BLK = DS * 128          # free extent of one [128, DS*128] DRAM block

    # block-layout params: [nblk, 128, DS*128]
    hsT = nc.declare_dram_parameter("hsT", [NQT, 128, BLK], F32R, isOutput=False)
    decT = nc.declare_dram_parameter("decT", [NKT, 128, BLK], F32R, isOutput=False)
    wq = nc.declare_dram_parameter("wq", [NDO, 128, BLK], F32R, isOutput=False)
    wkv = nc.declare_dram_parameter("wkv", [2 * NDO, 128, BLK], F32R, isOutput=False)
    out = nc.declare_dram_parameter("out", [QS, Dp], F32, isOutput=True)

    def load_blocks(dst, src, blk0, nblk):
        """DMA nblk consecutive [128, DS, 128] blocks into dst[:, :, j*128:...]."""
        for j in range(nblk):
            nc.sync.dma_start(
                dst[:, :, j * 128 : (j + 1) * 128],
                src[blk0 + j].rearrange("p (s o) -> p s o", o=128),
            )

    with tile.TileContext(nc) as tc:
        # Two SBUF allocation stacks: long-lived pools (identity, KT, V,
        # q-tiles, attention working set) go on the RIGHT stack and close at
        # the end; transient per-phase + prefetch pools go on the LEFT stack
        # and close in LIFO order at phase boundaries.
        pools = []

        def enter(cm):
            pools.append(cm)
            return cm.__enter__()

        def close(cm):
            pools.remove(cm)
            cm.__exit__(None, None, None)

        constp_cm = tc.tile_pool(name="const", bufs=1, side="right")
        dramp_cm = tc.tile_pool(name="dram", bufs=1, space="DRAM")
        whip_cm = tc.tile_pool(name="whi", bufs=NDC)
        wlop_cm = tc.tile_pool(name="wlo", bufs=NDO)
        dt1p_cm = tc.tile_pool(name="dt1", bufs=2)
        wqp_cm = tc.tile_pool(name="wqp", bufs=NDO)
        htp_cm = tc.tile_pool(name="hst", bufs=2)
        stgp_cm = tc.tile_pool(name="stg", bufs=4)
        psB_cm = tc.tile_pool(name="psB", bufs=3, space="PSUM")

        constp = enter(constp_cm)
        dramp = enter(dramp_cm)
        whip = enter(whip_cm)
        wlop = enter(wlop_cm)
        dt1p = enter(dt1p_cm)
        wqp = enter(wqp_cm)
        htp = enter(htp_cm)
        stgp = enter(stgp_cm)
        psB = enter(psB_cm)

        ident = constp.tile([128, 128], F32)
        nc.gpsimd.memset(ident[:], 0.0)
        nc.gpsimd.affine_select(
            out=ident[:], in_=ident[:],
            compare_op=mybir.AluOpType.not_equal,
            fill=1.0, base=0, pattern=[[-1, 128]], channel_multiplier=1,
        )
        qt_dram = dramp.tile([NQT, 128, BLK], F32R)

        # ---- phase-B-critical loads first, then background prefetch ----
        wqt = []
        for do in range(NDO):
            t = wqp.tile([128, DS, 128], F32R, tag="wqp", name=f"wq{do}")
            load_blocks(t, wq, do, 1)
            wqt.append(t)
        hts = []
        ht0 = htp.tile([128, DS, 512], F32R, tag="hst", name="ht0")
        load_blocks(ht0, hsT, 0, 4)
        hts.append(ht0)
        # prefetch for later phases (completes under B's compute)
        whis = []
        for j in range(NDC):
            t = whip.tile([128, DS, 512], F32R, tag="whi", name=f"whi{j}")
            load_blocks(t, wkv, NDO + 4 * j, 4)
            whis.append(t)
        wlo = []
        for do in range(NDO):
            t = wlop.tile([128, DS, 128], F32R, tag="wlo", name=f"wlo{do}")
            load_blocks(t, wkv, do, 1)
            wlo.append(t)
        dt1s = {}
        for kc in range(min(2, NA1)):
            t = dt1p.tile([128, DS, 512], F32R, tag="dt1", name=f"dt1_{kc}")
            load_blocks(t, decT, 4 * kc, 4)
            dt1s[kc] = t

        # ---------------- Phase B: QT[do, q] = Wq^T @ hsT -> DRAM -----------
        for qc in range(NQC):
            if qc + 1 < NQC:
                ht = htp.tile([128, DS, 512], F32R, tag="hst", name=f"ht{qc+1}")
                load_blocks(ht, hsT, 4 * (qc + 1), 4)
                hts.append(ht)
            for do in range(NDO):
                ps = psB.tile([128, 512], F32, tag="psB")
                for di in range(DS):
                    nc.tensor.matmul(
                        ps[:], wqt[do][:, di, :], hts[qc][:, di, :],
                        start=(di == 0), stop=(di == DS - 1),
                    )
                st = stgp.tile([128, 512], F32R, tag="stg")
                nc.vector.tensor_copy(st[:], ps[:])
                for half in range(4):
                    qt = 4 * qc + half
                    nc.sync.dma_start(
                        qt_dram[qt, :, do * 128 : (do + 1) * 128],
                        st[:, half * 128 : (half + 1) * 128],
                    )
        close(psB_cm)
        close(stgp_cm)
        close(htp_cm)
        close(wqp_cm)
        # left stack now: whip, wlop, dt1p

        # ---------------- Phase A1: KT[do, k] = Wkv_lo^T @ decT -------------
        ktp_cm = tc.tile_pool(name="ktp", bufs=1, side="right")
        qtp_cm = tc.tile_pool(name="qt", bufs=3, side="right")
        psA_cm = tc.tile_pool(name="psA", bufs=3, space="PSUM")
        ktp = enter(ktp_cm)
        qtp = enter(qtp_cm)
        psA = enter(psA_cm)
        KT = ktp.tile([128, DS, KLp], F32R, tag="KT")   # [d, k] rhs for scores

        # prefetch first attention q-tiles (qt_dram already fully written)
        qtiles = {}
        for qt in range(min(2, NQT)):
            qtile = qtp.tile([128, DS, 128], F32R, tag="qt", name=f"qtile{qt}")
            nc.sync.dma_start(
                qtile[:],
                qt_dram[qt].rearrange("p (s o) -> p s o", o=128),
            )
            qtiles[qt] = qtile

        for kc in range(NA1):
            if kc + 2 < NA1:
                t = dt1p.tile([128, DS, 512], F32R, tag="dt1", name=f"dt1_{kc+2}")
                load_blocks(t, decT, 4 * (kc + 2), 4)
                dt1s[kc + 2] = t
            dt = dt1s[kc]
            for do in range(NDO):
                ps = psA.tile([128, 512], F32, tag="psA")
                for di in range(DS):
                    nc.tensor.matmul(
                        ps[:], wlo[do][:, di, :], dt[:, di, :],
                        start=(di == 0), stop=(di == DS - 1),
                    )
                nc.vector.tensor_copy(
                    KT[:, do, kc * 512 : (kc + 1) * 512], ps[:]
                )
        close(psA_cm)
        close(dt1p_cm)
        close(wlop_cm)
        # left stack now: whip

        # ---------------- Phase A2: V[k, d] = decT^T @ Wkv_hi ---------------
        vp_cm = tc.tile_pool(name="vp", bufs=1, side="right")
        dt2p_cm = tc.tile_pool(name="dt2", bufs=2)
        psV_cm = tc.tile_pool(name="psV", bufs=3, space="PSUM")
        vp = enter(vp_cm)
        dt2p = enter(dt2p_cm)
        psV = enter(psV_cm)
        V = vp.tile([128, NKT, Dp], F32R, tag="V")       # [k, d] rhs for AV

        dt2s = {}
        for kt in range(min(2, NKT)):
            t = dt2p.tile([128, DS, 128], F32R, tag="dt2", name=f"dt2_{kt}")
            load_blocks(t, decT, kt, 1)
            dt2s[kt] = t
        for kt in range(NKT):
            if kt + 2 < NKT:
                t = dt2p.tile([128, DS, 128], F32R, tag="dt2", name=f"dt2_{kt+2}")
                load_blocks(t, decT, kt + 2, 1)
                dt2s[kt + 2] = t
            dt = dt2s[kt]
            for dc in range(NDC):
                ps = psV.tile([128, 512], F32, tag="psV")
                for di in range(DS):
                    nc.tensor.matmul(
                        ps[:], dt[:, di, :], whis[dc][:, di, :],
                        start=(di == 0), stop=(di == DS - 1),
                    )
                nc.vector.tensor_copy(
                    V[:, kt, dc * 512 : (dc + 1) * 512], ps[:]
                )
        close(psV_cm)
        close(dt2p_cm)
        close(whip_cm)

        # ---------------- Phase C: attention per q-tile ---------------------
        pp_cm = tc.tile_pool(name="pp", bufs=2, side="right")
        ptp_cm = tc.tile_pool(name="ptp", bufs=2, side="right")
        statp_cm = tc.tile_pool(name="stat", bufs=NQT, side="right")
        ostp_cm = tc.tile_pool(name="ost", bufs=2, side="right")
        ps_sc_cm = tc.tile_pool(name="ps_sc", bufs=4, space="PSUM")
        ps_pt_cm = tc.tile_pool(name="ps_pt", bufs=2, space="PSUM")
        ps_av_cm = tc.tile_pool(name="ps_av", bufs=2, space="PSUM")
        pp = enter(pp_cm)
        ptp = enter(ptp_cm)
        statp = enter(statp_cm)
        ostp = enter(ostp_cm)
        ps_sc = enter(ps_sc_cm)
        ps_pt = enter(ps_pt_cm)
        ps_av = enter(ps_av_cm)

        for qt in range(NQT):
            if qt + 2 < NQT:
                qtile = qtp.tile([128, DS, 128], F32R, tag="qt", name=f"qtile{qt+2}")
                nc.sync.dma_start(
                    qtile[:],
                    qt_dram[qt + 2].rearrange("p (s o) -> p s o", o=128),
                )
                qtiles[qt + 2] = qtile
            qtile = qtiles[qt]

            P = pp.tile([128, KLp], F32R, tag="pp")
            lpart = statp.tile([128, NKC + 1], F32, tag="stat")
            for kc in range(NKC):
                ps = ps_sc.tile([128, 512], F32, tag="ps_sc")
                for di in range(DS):
                    nc.tensor.matmul(
                        ps[:], qtile[:, di, :],
                        KT[:, di, kc * 512 : (kc + 1) * 512],
                        start=(di == 0), stop=(di == DS - 1),
                    )
                nc.scalar.activation(
                    P[:, kc * 512 : (kc + 1) * 512], ps[:], EXP,
                    bias=0.0, scale=float(scale),
                    accum_out=lpart[:, kc : kc + 1],
                )
            nc.vector.tensor_tensor(
                lpart[:, NKC : NKC + 1], lpart[:, 0:1], lpart[:, 1:2],
                mybir.AluOpType.add,
            )
            for kc in range(2, NKC):
                nc.vector.tensor_tensor(
                    lpart[:, NKC : NKC + 1], lpart[:, NKC : NKC + 1],
                    lpart[:, kc : kc + 1], mybir.AluOpType.add,
                )
            recip = statp.tile([128, 1], F32, tag="recip")
            nc.vector.reciprocal(recip[:], lpart[:, NKC : NKC + 1])

            PT = ptp.tile([128, NKT, 128], F32R, tag="ptp")
            avs = [
                ps_av.tile([128, 512], F32, tag="ps_av", name=f"av{i}")
                for i in range(NDC)
            ]
            for kt in range(NKT):
                pst = ps_pt.tile([128, 128], F32, tag="ps_pt")
                nc.tensor.transpose(
                    pst[:], P[:, kt * 128 : (kt + 1) * 128].bitcast(F32),
                    ident[:],
                )
                nc.vector.tensor_copy(PT[:, kt, :], pst[:])
                for dc in range(NDC):
                    nc.tensor.matmul(
                        avs[dc][:], PT[:, kt, :],
                        V[:, kt, dc * 512 : (dc + 1) * 512],
                        start=(kt == 0), stop=(kt == NKT - 1),
                    )
            ot = ostp.tile([128, Dp], F32, tag="ost")
            for dc in range(NDC):
                nc.vector.tensor_scalar(
                    ot[:, dc * 512 : (dc + 1) * 512], avs[dc][:],
                    recip[:], None, mybir.AluOpType.mult,
                )
            nc.sync.dma_start(out[qt * 128 : (qt + 1) * 128, :], ot[:])

        for cm in list(reversed(pools)):
            close(cm)


    legalize_waits(nc)
    return nc


def _pack_blocks(x):
    """[N, Dp] -> [Dp//128, 128, N] block layout:
    result[blk, p, n] = x[n, blk*128 + p]  (d on partitions, per-block)."""
    N, Dp = x.shape
    return np.ascontiguousarray(x.T.reshape(Dp // 128, 128, N))


def _pack_dT_blocks(x, DS):
    """[N, Dp] -> [N//128, 128, DS*128] where block b holds
    res[b, p, s*128+o] = x[b*128+o, s*128+p]  (the lhsT/rhs block layout:
    partitions carry d (inner 128 of subtile s), free carries (s, n-within-block)."""
    N, Dp = x.shape
    # [N, Dp] -> [nblk, 128(n), DS, 128(p)] -> transpose to [nblk, 128(p), DS, 128(n)]
    r = x.reshape(N // 128, 128, DS, 128).transpose(0, 3, 2, 1)
    return np.ascontiguousarray(r.reshape(N // 128, 128, DS * 128))


def prepare_in_maps(hidden_states, decoder_hidden_states, Wq, Wkv):
    hidden_states = np.asarray(hidden_states, dtype=np.float32)
    decoder_hidden_states = np.asarray(decoder_hidden_states, dtype=np.float32)
    Wq = np.asarray(Wq, dtype=np.float32)
    Wkv = np.asarray(Wkv, dtype=np.float32)
    QS = QL // 2
    DS = D // 128

    # weight block layouts: block do holds [128 p(=d_in within subtile), DS*128]
    # wq[do][p, s*128+o] = Wq[s*128+p, do*128+o]
    wq_p = _pack_dT_blocks(Wq.T, DS)      # Wq.T is [D_out, D_in]
    wkv_p = _pack_dT_blocks(Wkv.T, DS)

    in_maps = []
    for c in range(N_CORES):
        b, h = c // 2, c % 2
        hs = hidden_states[b, h * QS : (h + 1) * QS]        # [QS, D]
        dec = decoder_hidden_states[b]                      # [KL, D]
        in_maps.append(
            {
                "hsT": _pack_dT_blocks(hs, DS),    # [NQT, 128, DS*128]
                "decT": _pack_dT_blocks(dec, DS),  # [NKT, 128, DS*128]
                "wq": wq_p,
                "wkv": wkv_p,
            }
        )
    return in_maps


def kernel(hidden_states, decoder_hidden_states, Wq, Wkv):
    QS = QL // 2
    scale = 1.0 / float(np.sqrt(D))

    nc = bass.Bass()
    build_attention(nc, QS, KL, D, scale)
    in_maps = prepare_in_maps(hidden_states, decoder_hidden_states, Wq, Wkv)

    res = run_bass_kernel_spmd(nc, in_maps, list(range(N_CORES)))

    out = np.empty((B, QL, D), dtype=np.float32)
    for c in range(N_CORES):
        b, h = c // 2, c % 2
        out[b, h * QS : (h + 1) * QS] = res.results[c]["out"]
    return out


# revision 19
# speedup vs baseline: 1.2049x; 1.2049x over previous
"""Trainium2 Bass kernel for CrossAttention (B=4, QL=KL=2048, D=1024, fp32).

reference:
    query = hidden_states @ Wq                      # [B, QL, D]
    kv    = decoder_hidden_states @ Wkv             # [B, KL, 2D]
    key, value = split(kv, 2, axis=-1)
    scores = einsum('bqd,bkd->bqk', query, key) / sqrt(D)
    w = softmax(scores, axis=-1)
    out = einsum('bqk,bkd->bqd', w, value)          # [B, QL, D]

Sharding: 8 cores = batch(4) x q-half(2).  Each core owns 1024 query rows of
one batch and computes the full K/V projection for its batch (KV work
duplicated x2 across the pair sharing a batch; no collectives needed).

All matmuls run in float32r (TF32-like), which streams at full PE rate for
moving dims >= 256.  Softmax runs without max-subtraction (scores here are
~N(0,1); exp stays far from fp32 limits) using ACT's fused exp(scale*x) with
accum_out row sums.  P^T for the AV matmul is built with DVE 32x32 stream
transposes (sbuf->sbuf), and the attention loop is software-pipelined so PE
runs scores(q+1) while DVE transposes P(q).

Phase order QT -> KT -> V -> attention.  SBUF is managed on two allocation
stacks (long-lived pools right, transient pools left) so later phases'
weights prefetch during earlier phases' compute.  DMA issue order is
critical-first: each phase's first-needed chunk is issued before background
prefetch, and bulk tensors move as single multi-block DMAs (one SWDGE
trigger, 4KB descriptor rows).

This walrus build allows only ONE embedded semaphore wait per hardware
instruction; legalize_waits() splits any extra waits onto injected
same-engine NOPs after Tile scheduling.
"""

import sys

if "/opt/trn_rl_repo" not in sys.path:
    sys.path.insert(0, "/opt/trn_rl_repo")

import numpy as np

import bass_rust
import concourse.bass as bass
import concourse.mybir as mybir
import concourse.tile as tile
from concourse.bass_utils import run_bass_kernel_spmd

F32 = mybir.dt.float32
F32R = mybir.dt.float32r
EXP = mybir.ActivationFunctionType.Exp

N_CORES = 8
B, QL, KL, D = 4, 2048, 2048, 1024


def legalize_waits(nc, max_waits=1):
    """TRN2 instructions embed at most one semaphore wait.  Move excess waits
    emitted by Tile onto same-engine NOPs inserted just before the owning
    instruction (engine FIFO makes this semantically identical)."""
    cnt = 0
    for fn in nc.m.functions:
        for bb in fn.blocks:
            out = []
            changed = False
            for ins in bb.instructions:
                si = ins.sync_info
                if si is not None and si.on_wait and len(si.on_wait) > max_waits:
                    waits = list(si.on_wait)
                    for w in waits[:-max_waits]:
                        cnt += 1
                        nop = bass_rust.InstNoOp(name=f"I-wfix-{cnt}")
                        nop.engine = ins.engine
                        nop.sync_info = mybir.SyncInfo(on_wait=[w], on_update=[])
                        out.append(nop)
                    ins.sync_info = mybir.SyncInfo(
                        on_wait=waits[-max_waits:],
                        on_update=list(si.on_update or []),
                    )
                    changed = True
                out.append(ins)
            if changed:
                bb.instructions = out
    return cnt


def build_attention(nc, QS, KLp, Dp, scale):
    DS = Dp // 128          # contraction subtiles
    NDO = Dp // 128         # output-d 128-chunks
    NKC = KLp // 512        # k 512-chunks (scores)
    NKT = KLp // 128        # k 128-chunks
    NQT = QS // 128         # q tiles
    NDC = Dp // 512         # d 512-chunks (AV / Wkv_hi)
    NA1 = KLp // 512        # A1 rhs 512-chunks
    NQC = QS // 512         # B rhs 512-chunks
    BLK = DS * 128          # free extent of one [128, DS*128] DRAM block

    # block-layout params: [nblk, 128, DS*128]
    hsT = nc.declare_dram_parameter("hsT", [NQT, 128, BLK], F32R, isOutput=False)
    decT = nc.declare_dram_parameter("decT", [NKT, 128, BLK], F32R, isOutput=False)
    wq = nc.declare_dram_parameter("wq", [NDO, 128, BLK], F32R, isOutput=False)
    wkv = nc.declare_dram_parameter("wkv", [2 * NDO, 128, BLK], F32R, isOutput=False)
    out = nc.declare_dram_parameter("out", [QS, Dp], F32, isOutput=True)

    def load_blocks(dst, src, blk0, nblk):
        """One DMA moving nblk consecutive [128, BLK] DRAM blocks into an
        SBUF tile laid out [128, DS, nblk, 128] (or [128, DS, 128] if 1)."""
        if nblk == 1:
            nc.sync.dma_start(
                dst[:], src[blk0].rearrange("p (s o) -> p s o", o=128)
            )
        else:
            nc.sync.dma_start(
                dst.rearrange("p b s o -> p b (s o)"),
                src[blk0 : blk0 + nblk].rearrange("b p f -> p b f"),
            )

    with tile.TileContext(nc) as tc:
        # Two SBUF allocation stacks: long-lived pools (identity, KT, V,
        # q-tiles, attention working set) on the RIGHT stack close at the
        # end; transient per-phase + prefetch pools on the LEFT stack close
        # LIFO at phase boundaries.
        pools = []

        def enter(cm):
            pools.append(cm)
            return cm.__enter__()

        def close(cm):
            pools.remove(cm)
            cm.__exit__(None, None, None)

        constp_cm = tc.tile_pool(name="const", bufs=1, side="right")
        dramp_cm = tc.tile_pool(name="dram", bufs=1, space="DRAM")
        whip_cm = tc.tile_pool(name="whi", bufs=1)
        wlop_cm = tc.tile_pool(name="wlo", bufs=1)
        dt1p_cm = tc.tile_pool(name="dt1", bufs=2)
        wqp_cm = tc.tile_pool(name="wqp", bufs=1)
        htp_cm = tc.tile_pool(name="hst", bufs=2)
        stgp_cm = tc.tile_pool(name="stg", bufs=4)
        psB_cm = tc.tile_pool(name="psB", bufs=3, space="PSUM")

        constp = enter(constp_cm)
        dramp = enter(dramp_cm)
        whip = enter(whip_cm)
        wlop = enter(wlop_cm)
        dt1p = enter(dt1p_cm)
        wqp = enter(wqp_cm)
        htp = enter(htp_cm)
        stgp = enter(stgp_cm)
        psB = enter(psB_cm)

        ident = constp.tile([128, 128], F32)
        nc.gpsimd.memset(ident[:], 0.0)
        nc.gpsimd.affine_select(
            out=ident[:], in_=ident[:],
            compare_op=mybir.AluOpType.not_equal,
            fill=1.0, base=0, pattern=[[-1, 128]], channel_multiplier=1,
        )
        qt_dram = dramp.tile([NQC, 128, DS, 512], F32R)

        # reserve the prefetch tiles up-front (left stack, stable addresses);
        # their DMAs are issued later, behind B's critical loads
        whi = whip.tile([128, NDO, DS, 128], F32R, tag="whi")
        wlo = wlop.tile([128, NDO, DS, 128], F32R, tag="wlo")
        dt1s = {}
        for kc in range(min(2, NA1)):
            dt1s[kc] = dt1p.tile([128, 4, DS, 128], F32R, tag="dt1", name=f"dt1_{kc}")

        # ---- critical-first loads: B's first groups, then the rest of wq ---
        wqt = wqp.tile([128, NDO, DS, 128], F32R, tag="wqp")
        load_blocks(wqt[:, 0:2], wq, 0, 2)
        hts = []
        ht0 = htp.tile([128, 4, DS, 128], F32R, tag="hst", name="ht0")
        load_blocks(ht0[:], hsT, 0, 4)
        hts.append(ht0)
        load_blocks(wqt[:, 2:NDO], wq, 2, NDO - 2)

        # ---------------- Phase B: QT[do, q] = Wq^T @ hsT -> DRAM -----------
        for qc in range(NQC):
            if qc + 1 < NQC:
                ht = htp.tile([128, 4, DS, 128], F32R, tag="hst", name=f"ht{qc+1}")
                load_blocks(ht[:], hsT, 4 * (qc + 1), 4)
                hts.append(ht)
            for do in range(NDO):
                if qc == NQC - 1:
                    # background prefetch for A1, spread across B's last wave
                    if do == 1:
                        load_blocks(wlo[:], wkv, 0, NDO)
                    elif do == 3 and 0 in dt1s:
                        load_blocks(dt1s[0][:], decT, 0, 4)
                    elif do == 5 and 1 in dt1s:
                        load_blocks(dt1s[1][:], decT, 4, 4)
                ps = psB.tile([128, 512], F32, tag="psB")
                for di in range(DS):
                    nc.tensor.matmul(
                        ps[:], wqt[:, do, di, :], hts[qc][:, :, di, :],
                        start=(di == 0), stop=(di == DS - 1),
                    )
                st = stgp.tile([128, 512], F32R, tag="stg")
                nc.vector.tensor_copy(st[:], ps[:])
                nc.sync.dma_start(qt_dram[qc, :, do, :], st[:])
        if NDO <= 5 and 1 in dt1s:
            # small-config catch-up: B's last wave had no do==5 slot
            load_blocks(dt1s[1][:], decT, 4, 4)
        close(psB_cm)
        close(stgp_cm)
        close(htp_cm)
        close(wqp_cm)

        # ---------------- Phase A1: KT[do, k] = Wkv_lo^T @ decT -------------
        ktp_cm = tc.tile_pool(name="ktp", bufs=1, side="right")
        qtp_cm = tc.tile_pool(name="qt", bufs=3, side="right")
        psA_cm = tc.tile_pool(name="psA", bufs=3, space="PSUM")
        ktp = enter(ktp_cm)
        qtp = enter(qtp_cm)
        psA = enter(psA_cm)
        KT = ktp.tile([128, DS, KLp], F32R, tag="KT")   # [d, k] rhs for scores
        qtiles = {}

        for kc in range(NA1):
            if kc + 2 < NA1:
                t = dt1p.tile([128, 4, DS, 128], F32R, tag="dt1", name=f"dt1_{kc+2}")
                load_blocks(t[:], decT, 4 * (kc + 2), 4)
                dt1s[kc + 2] = t
            if kc == 1:
                # prefetch A2's weights under A1's compute
                load_blocks(whi[:], wkv, NDO, NDO)
            dt = dt1s[kc]
            for do in range(NDO):
                ps = psA.tile([128, 512], F32, tag="psA")
                for di in range(DS):
                    nc.tensor.matmul(
                        ps[:], wlo[:, do, di, :], dt[:, :, di, :],
                        start=(di == 0), stop=(di == DS - 1),
                    )
                nc.vector.tensor_copy(
                    KT[:, do, kc * 512 : (kc + 1) * 512], ps[:]
                )
        close(psA_cm)
        close(dt1p_cm)
        close(wlop_cm)

        # ---------------- Phase A2: V[k, d] = decT^T @ Wkv_hi ---------------
        vp_cm = tc.tile_pool(name="vp", bufs=1, side="right")
        dt2p_cm = tc.tile_pool(name="dt2", bufs=3)
        psV_cm = tc.tile_pool(name="psV", bufs=3, space="PSUM")
        vp = enter(vp_cm)
        dt2p = enter(dt2p_cm)
        psV = enter(psV_cm)
        V = vp.tile([128, NKT, Dp], F32R, tag="V")       # [k, d] rhs for AV

        dt2s = {}
        for kt in range(min(3, NKT)):
            t = dt2p.tile([128, DS, 128], F32R, tag="dt2", name=f"dt2_{kt}")
            load_blocks(t, decT, kt, 1)
            dt2s[kt] = t
        for kt in range(NKT):
            if kt + 3 < NKT:
                t = dt2p.tile([128, DS, 128], F32R, tag="dt2", name=f"dt2_{kt+3}")
                load_blocks(t, decT, kt + 3, 1)
                dt2s[kt + 3] = t
            if kt == NKT - 2:
                # prefetch first attention q-tiles (qt_dram fully written)
                for qt in range(min(2, NQT)):
                    qtile = qtp.tile(
                        [128, DS, 128], F32R, tag="qt", name=f"qtile{qt}"
                    )
                    nc.sync.dma_start(
                        qtile[:],
                        qt_dram[qt // 4][:, :, (qt % 4) * 128 : (qt % 4 + 1) * 128],
                    )
                    qtiles[qt] = qtile
            dt = dt2s[kt]
            for dc in range(NDC):
                ps = psV.tile([128, 512], F32, tag="psV")
                for di in range(DS):
                    nc.tensor.matmul(
                        ps[:], dt[:, di, :], whi[:, 4 * dc : 4 * (dc + 1), di, :],
                        start=(di == 0), stop=(di == DS - 1),
                    )
                nc.vector.tensor_copy(
                    V[:, kt, dc * 512 : (dc + 1) * 512], ps[:]
                )
        close(psV_cm)
        close(dt2p_cm)
        close(whip_cm)

        # ---------------- Phase C: attention per q-tile ---------------------
        pp_cm = tc.tile_pool(name="pp", bufs=2, side="right")
        ptp1_cm = tc.tile_pool(name="ptp1", bufs=1, side="right")
        ptp_cm = tc.tile_pool(name="ptp", bufs=2, side="right")
        statp_cm = tc.tile_pool(name="stat", bufs=NQT, side="right")
        ostp_cm = tc.tile_pool(name="ost", bufs=2, side="right")
        ps_sc_cm = tc.tile_pool(name="ps_sc", bufs=5, space="PSUM")
        ps_av_cm = tc.tile_pool(name="ps_av", bufs=3, space="PSUM")
        pp = enter(pp_cm)
        ptp1 = enter(ptp1_cm)
        ptp = enter(ptp_cm)
        statp = enter(statp_cm)
        ostp = enter(ostp_cm)
        ps_sc = enter(ps_sc_cm)
        ps_av = enter(ps_av_cm)

        def emit_scores(qt):
            """scores + exp + row-sum stats for q-tile qt."""
            qtile = qtiles[qt]
            P = pp.tile([128, NKT, 128], F32, tag="pp", name=f"P{qt}")
            lpart = statp.tile([128, NKC + 1], F32, tag="stat", name=f"lp{qt}")
            for kc in range(NKC):
                ps = ps_sc.tile([128, 512], F32, tag="ps_sc")
                for di in range(DS):
                    nc.tensor.matmul(
                        ps[:], qtile[:, di, :],
                        KT[:, di, kc * 512 : (kc + 1) * 512],
                        start=(di == 0), stop=(di == DS - 1),
                    )
                nc.scalar.activation(
                    P[:, 4 * kc : 4 * (kc + 1), :], ps[:], EXP,
                    bias=0.0, scale=float(scale),
                    accum_out=lpart[:, kc : kc + 1],
                )
            return P, lpart

        def emit_softmax_stats(lpart, qt):
            nc.vector.tensor_tensor(
                lpart[:, NKC : NKC + 1], lpart[:, 0:1], lpart[:, 1:2],
                mybir.AluOpType.add,
            )
            for kc in range(2, NKC):
                nc.vector.tensor_tensor(
                    lpart[:, NKC : NKC + 1], lpart[:, NKC : NKC + 1],
                    lpart[:, kc : kc + 1], mybir.AluOpType.add,
                )
            recip = statp.tile([128, 1], F32, tag="recip", name=f"rc{qt}")
            nc.vector.reciprocal(recip[:], lpart[:, NKC : NKC + 1])
            return recip

        def emit_transposes(P, qt):
            """PT[k, kt, q] = P[q, kt, k].T per kt: DVE 32x32 stream blocks
            (f32), then one rounding copy to f32r for the AV matmul."""
            PT1 = ptp1.tile([128, NKT, 128], F32, tag="ptp1", name=f"PT1_{qt}")
            for a in range(4):
                for c in range(4):
                    nc.vector.transpose(
                        PT1[32 * c : 32 * c + 32, :, 32 * a : 32 * a + 32],
                        P[32 * a : 32 * a + 32, :, 32 * c : 32 * c + 32],
                    )
            PT = ptp.tile([128, NKT, 128], F32R, tag="ptp", name=f"PT{qt}")
            nc.vector.tensor_copy(PT[:], PT1[:])
            return PT

        def emit_av(qt, PT, recip):
            avs = [
                ps_av.tile([128, 512], F32, tag="ps_av", name=f"av{qt}_{i}")
                for i in range(NDC)
            ]
            for kt in range(NKT):
                for dc in range(NDC):
                    nc.tensor.matmul(
                        avs[dc][:], PT[:, kt, :],
                        V[:, kt, dc * 512 : (dc + 1) * 512],
                        start=(kt == 0), stop=(kt == NKT - 1),
                    )
            ot = ostp.tile([128, Dp], F32, tag="ost")
            for dc in range(NDC):
                nc.vector.tensor_scalar(
                    ot[:, dc * 512 : (dc + 1) * 512], avs[dc][:],
                    recip[:], None, mybir.AluOpType.mult,
                )
            nc.sync.dma_start(out[qt * 128 : (qt + 1) * 128, :], ot[:])

        # software pipeline: PE runs scores(q+1) while DVE transposes P(q)
        state = {}
        for qt in range(NQT):
            if qt + 2 < NQT:
                qtile = qtp.tile([128, DS, 128], F32R, tag="qt", name=f"qtile{qt+2}")
                nc.sync.dma_start(
                    qtile[:],
                    qt_dram[(qt + 2) // 4][
                        :, :, ((qt + 2) % 4) * 128 : ((qt + 2) % 4 + 1) * 128
                    ],
                )
                qtiles[qt + 2] = qtile
            P, lpart = emit_scores(qt)
            recip = emit_softmax_stats(lpart, qt)
            PT = emit_transposes(P, qt)
            state[qt] = (PT, recip)
            if qt > 0:
                emit_av(qt - 1, *state.pop(qt - 1))
        emit_av(NQT - 1, *state.pop(NQT - 1))

        for cm in list(reversed(pools)):
            close(cm)

    legalize_waits(nc)
    return nc


def _pack_dT_blocks(x, DS):
    """[N, Dp] -> [N//128, 128, DS*128] where block b holds
    res[b, p, s*128+o] = x[b*128+o, s*128+p]  (partitions carry d, free
    carries (subtile s, n-within-block))."""
    N, Dp = x.shape
    r = x.reshape(N // 128, 128, DS, 128).transpose(0, 3, 2, 1)
    return np.ascontiguousarray(r.reshape(N // 128, 128, DS * 128))


def prepare_in_maps(hidden_states, decoder_hidden_states, Wq, Wkv):
    hidden_states = np.asarray(hidden_states, dtype=np.float32)
    decoder_hidden_states = np.asarray(decoder_hidden_states, dtype=np.float32)
    Wq = np.asarray(Wq, dtype=np.float32)
    Wkv = np.asarray(Wkv, dtype=np.float32)
    QS = QL // 2
    DS = D // 128

    wq_p = _pack_dT_blocks(Wq.T, DS)      # [do][p, s*128+o] = Wq[s*128+p, do*128+o]
    wkv_p = _pack_dT_blocks(Wkv.T, DS)

    in_maps = []
    for c in range(N_CORES):
        b, h = c // 2, c % 2
        hs = hidden_states[b, h * QS : (h + 1) * QS]        # [QS, D]
        dec = decoder_hidden_states[b]                      # [KL, D]
        in_maps.append(
            {
                "hsT": _pack_dT_blocks(hs, DS),    # [NQT, 128, DS*128]
                "decT": _pack_dT_blocks(dec, DS),  # [NKT, 128, DS*128]
                "wq": wq_p,
                "wkv": wkv_p,
            }
        )
    return in_maps


def kernel(hidden_states, decoder_hidden_states, Wq, Wkv):
    QS = QL // 2
    scale = 1.0 / float(np.sqrt(D))

    nc = bass.Bass()
    build_attention(nc, QS, KL, D, scale)
    in_maps = prepare_in_maps(hidden_states, decoder_hidden_states, Wq, Wkv)

    res = run_bass_kernel_spmd(nc, in_maps, list(range(N_CORES)))

    out = np.empty((B, QL, D), dtype=np.float32)
    for c in range(N_CORES):
        b, h = c // 2, c % 2
        out[b, h * QS : (h + 1) * QS] = res.results[c]["out"]
    return out
